# revision 1
# baseline (speedup 1.0000x reference)
"""HarmonicSynth Trainium kernel: 8-way (batch x time-half) data-parallel.

Host does O(L) prep (exact f32 replication of the reference's linear
interpolation + an f64 prefix-sum of the fundamental phase, shipped as
wrapped "turns"); the device does all per-(sample, harmonic) work:
angle construction + range reduction, sin, anti-alias masking, and the
harmonic-weighted accumulation.
"""
import sys

import numpy as np

for _p in ("/opt/trn_rl_repo", "/root/.axon_site/_ro/trn_rl_repo"):
    try:
        import concourse  # noqa: F401
        break
    except ImportError:
        if _p not in sys.path:
            sys.path.insert(0, _p)

SR = 48000
NH = 60
T = 1000
HOP = 192
L = T * HOP          # 192000
B = 4
NCORES = 8
FPC = 500            # frames per core (time-half)
TILES = 4            # tiles per core
TF = 125             # frames per tile
PI = float(np.pi)
TWO_PI = float(2.0 * np.pi)
MAGIC = float(2 ** 23)
AA_LIM = float(SR * 0.49)   # 23520.0
H_MASK_MIN = 48      # smallest h for which f0*h can reach AA_LIM

_CACHE = {}


def _host_prep(f0, amplitudes, harmonic_distribution):
    f32 = np.float32
    f0 = np.asarray(f0, dtype=f32).reshape(B, T)
    amp = np.asarray(amplitudes, dtype=f32).reshape(B, T)
    harm = np.asarray(harmonic_distribution, dtype=f32).reshape(B, T, NH)

    # exact f32 replication of the reference's interpolation grid
    pos = np.maximum((np.arange(L, dtype=f32) + f32(0.5)) * f32(T / L) - f32(0.5), f32(0.0))
    i0 = np.minimum(np.floor(pos).astype(np.int64), T - 1)
    i1 = np.minimum(i0 + 1, T - 1)
    w = (pos - i0.astype(f32)).astype(f32)

    f0up = (f0[:, i0] * (f32(1.0) - w)[None, :] + f0[:, i1] * w[None, :]).astype(f32)
    ampup = (amp[:, i0] * (f32(1.0) - w)[None, :] + amp[:, i1] * w[None, :]).astype(f32)
    ampeff = (ampup * (f0up > 0).astype(f32)).astype(f32)

    # fundamental phase, f64 prefix sum, shipped wrapped in "turns"
    ph64 = np.cumsum(f0up.astype(np.float64) * (1.0 / SR), axis=1)
    u = np.mod(ph64, 1.0).astype(f32)                      # (B, L) in [0,1)

    # per-sample interp weight for the harm/accB term
    left = (np.arange(L) % HOP) < (HOP // 2)
    wtj = np.where(left, w - f32(1.0), w).astype(f32)      # (L,)
    wtj = np.broadcast_to(wtj, (B, L)).astype(f32)

    # per-frame harmonic coefficient tables
    cA = harm
    cBL = np.zeros_like(harm)
    cBL[:, 1:] = harm[:, 1:] - harm[:, :-1]
    cBR = np.zeros_like(harm)
    cBR[:, :-1] = harm[:, 1:] - harm[:, :-1]

    in_maps = []
    for core in range(NCORES):
        b, half = core // 2, core % 2
        sl = slice(half * L // 2, (half + 1) * L // 2)
        fsl = slice(half * FPC, (half + 1) * FPC)
        in_maps.append({
            "u": np.ascontiguousarray(u[b, sl].reshape(FPC, HOP)),
            "f0up": np.ascontiguousarray(f0up[b, sl].reshape(FPC, HOP)),
            "ampeff": np.ascontiguousarray(ampeff[b, sl].reshape(FPC, HOP)),
            "wtj": np.ascontiguousarray(wtj[b, sl].reshape(FPC, HOP)),
            "cA": np.ascontiguousarray(cA[b, fsl]),
            "cBL": np.ascontiguousarray(cBL[b, fsl]),
            "cBR": np.ascontiguousarray(cBR[b, fsl]),
        })
    return in_maps


def _register_frac_op():
    """out = (t - round(t)) * ((in1*s0) < imm2), t = in0*s0.
    Round-to-nearest via the +-2^23 magic add; imm2 is the AA limit
    (or FLT_MAX for unmasked harmonics)."""
    if "fracop" in _CACHE:
        return _CACHE["fracop"]
    import numpy as np
    import concourse.dve_ops as dops
    from concourse.dve_spec import Spec, Src0, Src1, C0, C1, C2

    t = Src0 * C0
    r = (t + C1) - C1
    body = (t - r) * ((Src1 * C0) < C2)

    def _ref(in0, in1, s0, s1, imm2):
        f = np.float32
        t = (in0.astype(f) * f(s0)).astype(f)
        r = ((t + f(s1)).astype(f) - f(s1)).astype(f)
        m = ((in1.astype(f) * f(s0)).astype(f) < f(imm2)).astype(f)
        return ((t - r).astype(f) * m).astype(f)

    def _register(op):
        dops.OPS.append(op)
        dops.CUSTOM_DVE_SPECS[op.name] = op.spec
        dops._SUB_OPCODE_FOR_NAME[op.name] = dops._CUSTOM_DVE_ROW_BASE + len(dops.OPS) - 1
        for ver in ("v3", "v4"):
            try:
                op.compile(ver)
            except ValueError as e:
                import re
                m = re.search(r"\(%s: ([0-9a-f]+)" % ver, str(e))
                if not m:
                    raise
                op.uops_sha[ver] = m.group(1)
                op.compile(ver)

    op = dops.DveOp("FRAC_MASK_ANT", Spec(body=body, reference=_ref),
                    subdim=False, uops_sha={})
    _register(op)

    # accB MAC with a left/right coefficient switch at Idx == imm2:
    # out = in0 * (Idx < imm2 ? s0 : s1) + in1
    from concourse.dve_spec import Idx
    body2 = Src0 * (C1 + (Idx < C2) * (C0 - C1)) + Src1

    def _ref2(in0, in1, s0, s1, imm2):
        f = np.float32
        idx = np.arange(in0.shape[-1], dtype=f)
        coef = np.where(idx[None, :] < f(imm2), s0, s1).astype(f)
        return ((in0.astype(f) * coef).astype(f) + in1.astype(f)).astype(f)

    op2 = dops.DveOp("MAC_LR_ANT", Spec(body=body2, reference=_ref2),
                     subdim=False, uops_sha={})
    _register(op2)
    _CACHE["fracop"] = (op, op2)
    return _CACHE["fracop"]


def _build_nc():
    if "nc" in _CACHE:
        return _CACHE["nc"]
    import concourse.bass as bass
    import concourse.bacc as bacc
    import concourse.tile as tile
    import concourse.mybir as mybir
    fracop, mac2op = _register_frac_op()

    A = mybir.AluOpType
    F32 = mybir.dt.float32
    nc = bacc.Bacc("TRN2", target_bir_lowering=False, debug=False, num_devices=NCORES)

    dr = {}
    for name, shape in [("u", [FPC, HOP]), ("f0up", [FPC, HOP]),
                        ("ampeff", [FPC, HOP]), ("wtj", [FPC, HOP]),
                        ("cA", [FPC, NH]), ("cBL", [FPC, NH]), ("cBR", [FPC, NH])]:
        dr[name] = nc.dram_tensor(name, shape, F32, kind="ExternalInput").ap()
    out_d = nc.dram_tensor("out", [FPC, HOP], F32, kind="ExternalOutput").ap()

    HH = HOP // 2
    with tile.TileContext(nc, trace_sim=False) as tc:
        with tc.tile_pool(name="io", bufs=TILES) as io_pool, \
             tc.tile_pool(name="coef", bufs=TILES) as coef_pool, \
             tc.tile_pool(name="acc", bufs=TILES) as acc_pool, \
             tc.tile_pool(name="work", bufs=8) as work_pool, \
             tc.tile_pool(name="cst", bufs=1) as cst_pool:
            twopi = cst_pool.tile([128, 1], F32)
            nc.vector.memset(twopi[:], TWO_PI)

            for t in range(TILES):
                rows = slice(t * TF, (t + 1) * TF)
                ut = io_pool.tile([TF, HOP], F32, tag="u")
                f0t = io_pool.tile([TF, HOP], F32, tag="f0")
                apt = io_pool.tile([TF, HOP], F32, tag="amp")
                wtt = io_pool.tile([TF, HOP], F32, tag="wt")
                nc.sync.dma_start(ut[:], dr["u"][rows, :])
                nc.sync.dma_start(f0t[:], dr["f0up"][rows, :])
                nc.sync.dma_start(apt[:], dr["ampeff"][rows, :])
                nc.sync.dma_start(wtt[:], dr["wtj"][rows, :])
                cat = coef_pool.tile([TF, NH], F32, tag="cA")
                cblt = coef_pool.tile([TF, NH], F32, tag="cBL")
                cbrt = coef_pool.tile([TF, NH], F32, tag="cBR")
                nc.sync.dma_start(cat[:], dr["cA"][rows, :])
                nc.sync.dma_start(cblt[:], dr["cBL"][rows, :])
                nc.sync.dma_start(cbrt[:], dr["cBR"][rows, :])

                accA = acc_pool.tile([TF, HOP], F32, tag="accA")
                accB = acc_pool.tile([TF, HOP], F32, tag="accB")

                for h in range(1, NH + 1):
                    fh = float(h)
                    fr = work_pool.tile([TF, HOP], F32, tag="f")
                    # fr = (u*h - round(u*h)) * aa_mask, one fused DVE op
                    lim = AA_LIM if h >= H_MASK_MIN else 3.0e38
                    nc.vector._custom_dve(fracop, out=fr[:], in0=ut[:], in1=f0t[:],
                                          s0=fh, s1=MAGIC, imm2=lim)
                    sn = work_pool.tile([TF, HOP], F32, tag="s")
                    # sin(2*pi*frac) == sin(h * 2*pi*u)  (masked -> sin(0) = 0)
                    nc.scalar.activation(sn[:], fr[:], mybir.ActivationFunctionType.Sin,
                                         scale=twopi[:TF, 0:1])
                    if h == 1:
                        nc.vector.tensor_scalar(accA[:], sn[:], cat[:, h - 1:h], None, A.mult)
                        nc.vector.tensor_scalar(accB[:, :HH], sn[:, :HH], cblt[:, h - 1:h], None, A.mult)
                        nc.vector.tensor_scalar(accB[:, HH:], sn[:, HH:], cbrt[:, h - 1:h], None, A.mult)
                    else:
                        nc.vector.scalar_tensor_tensor(accA[:], sn[:], cat[:, h - 1:h], accA[:],
                                                       A.mult, A.add)
                        nc.vector._custom_dve(mac2op, out=accB[:], in0=sn[:], in1=accB[:],
                                              s0=cblt[:, h - 1:h], s1=cbrt[:, h - 1:h],
                                              imm2=float(HH))

                # mono = (accA + wtj*accB) * ampeff
                nc.vector.tensor_tensor(accB[:], accB[:], wtt[:], A.mult)
                nc.vector.tensor_tensor(accA[:], accA[:], accB[:], A.add)
                nc.vector.tensor_tensor(accA[:], accA[:], apt[:], A.mult)
                nc.sync.dma_start(out_d[rows, :], accA[:])
    nc.compile()
    _CACHE["nc"] = nc
    return nc


def _run(in_maps, trace=False):
    from concourse.bass_utils import run_bass_kernel_spmd
    nc = _build_nc()
    try:
        return run_bass_kernel_spmd(nc, in_maps, list(range(NCORES)), trace=trace)
    except ModuleNotFoundError:
        return run_bass_kernel_spmd(nc, in_maps, list(range(NCORES)), trace=False)


def kernel(f0, amplitudes, harmonic_distribution, _trace=False, _want_res=False):
    in_maps = _host_prep(f0, amplitudes, harmonic_distribution)
    res = _run(in_maps, trace=_trace)
    out = np.empty((B, L), dtype=np.float32)
    for core in range(NCORES):
        b, half = core // 2, core % 2
        out[b, half * L // 2:(half + 1) * L // 2] = res.results[core]["out"].reshape(-1)
    if _want_res:
        return out, res
    return out

